# revision 1
# baseline (speedup 1.0000x reference)
"""Encoder-decoder LSTM seq2seq loss kernel for 8 TRN2 NeuronCores.

Strategy:
  - LSTM recurrences (encoder 48 steps, decoder 47 steps) are replicated on
    every core in gate-major layout: gates^T [2048, 64] computed as 16
    [128,64] PSUM chunks, state kept transposed (hT [128, 4*64]) so no
    per-step transposes are needed.
  - Input-side gate contributions (x @ W_ih^T + b) are batched in 8-step
    windows as full-utilization [128,128]x[128,512] matmuls, and the
    window matmuls are interleaved between recurrence steps so they fill
    PE idle gaps. The per-step x-injection into the gates PSUM is done by
    the PE itself (identity matmul, exact for 1.0*bf16) so the critical
    h-chain has no extra DVE hop.
  - Gates PSUM is split into three tiles [i|f], [g], [o] with the g
    chunks issued first so the c-path (tanh g, c update, tanh c) runs
    under the remaining matmuls; the o chunks are issued last so the
    h tail is just sigmoid(o) * tanh(c).
  - The 47 decoder logit matmuls are deferred until after the recurrence
    (the loss does not feed back) and run as one big GEMM against the
    core's 4000-row vocab shard (padded to 4096), step*batch-major, so the
    softmax denominator falls out of the ACT Exp instruction's free-axis
    accumulator for free.
  - Target logits come from a host-pregathered W_out[tgt] (dot with h via
    DVE multiply + ones-matmul contraction).
  - Host combines per-core partial sum-exp + target logits into the scalar
    loss (tiny: 8 x [128,24] + [1,3008]).
"""

import sys

sys.path.insert(0, "/opt/trn_rl_repo")

import numpy as np
import ml_dtypes

BF16 = ml_dtypes.bfloat16

# Model dims (hardcoded per contract)
SRC, TGT, B, H, V = 48, 48, 64, 512, 32000
DEC = TGT - 1                  # 47 decoder steps
SB = DEC * B                   # 3008 (step*batch)
SBC = 24                       # ceil(3008/128) sb-chunks
SBP = SBC * 128                # 3072 padded
NCORES = 8
VSH = V // NCORES              # 4000 vocab rows per core
VSP = 4096                     # padded shard
WIN = 8                        # bulk x-part window (steps)
NG = 16                        # gate chunks (2048/128)
KC = 4                         # hidden chunks (512/128)

# gate-chunk indices in the permuted [i f o g] weight layout
I_CH = list(range(0, 4))
F_CH = list(range(4, 8))
O_CH = list(range(8, 12))
G_CH = list(range(12, 16))

_COMPILED = None


def _build():
    import concourse.bass as bass
    import concourse.bacc as bacc
    import concourse.tile as tile
    from concourse import mybir

    f32 = mybir.dt.float32
    bf16 = mybir.dt.bfloat16
    AF = mybir.ActivationFunctionType

    nc = bacc.Bacc("TRN2", target_bir_lowering=False, debug=False,
                   num_devices=NCORES)

    def din(name, shape, dt=bf16):
        return nc.dram_tensor(name, shape, dt, kind="ExternalInput").ap()

    xt_enc = din("xt_enc", [H, SRC * B])
    xt_dec = din("xt_dec", [H, SB])
    wi_e = din("wi_e", [KC, 128, 4 * H])
    wh_e = din("wh_e", [KC, 128, 4 * H])
    wi_d = din("wi_d", [KC, 128, 4 * H])
    wh_d = din("wh_d", [KC, 128, 4 * H])
    bias_e = din("bias_e", [128, NG], f32)
    bias_d = din("bias_d", [128, NG], f32)
    mask_in = din("mask", [SRC, 128, KC * B], mybir.dt.uint8)
    ident_in = din("ident", [128, 128])
    wot_in = din("wot", [KC, 128, VSP])
    bout_in = din("bout", [128, VSP])
    wtgt_in = din("wtgt", [KC, 128, SB])

    out_s = nc.dram_tensor("out_s", [128, SBC], f32, kind="ExternalOutput").ap()
    out_l = nc.dram_tensor("out_l", [1, SB], f32, kind="ExternalOutput").ap()

    with tile.TileContext(nc) as tc:
        from contextlib import ExitStack
        with ExitStack() as ctx:
            # ---- pools ----
            pconst = ctx.enter_context(tc.tile_pool(name="const", bufs=1))
            pht = ctx.enter_context(tc.tile_pool(name="ht", bufs=1))
            pgx = ctx.enter_context(tc.tile_pool(name="gx", bufs=2))
            pw = ctx.enter_context(tc.tile_pool(name="w", bufs=1))
            pxt = ctx.enter_context(tc.tile_pool(name="xtw", bufs=2))
            pstate = ctx.enter_context(tc.tile_pool(name="state", bufs=3))
            pact = ctx.enter_context(tc.tile_pool(name="act", bufs=2))
            pmask = ctx.enter_context(tc.tile_pool(name="mask", bufs=2))
            # (log pool is small: prod + sh tiles in logits phase)
            plog = ctx.enter_context(tc.tile_pool(name="log", bufs=2))

            # ---- constants ----
            def dve_const(src_ap, shape, dt, tag):
                dma_t = pconst.tile(shape, dt, tag=f"{tag}_dma")
                nc.sync.dma_start(dma_t[:], src_ap)
                t = pconst.tile(shape, dt, tag=tag)
                nc.vector.tensor_copy(t[:], dma_t[:])
                return t

            bias_e_t = dve_const(bias_e[:], [128, NG], f32, "be")
            bias_d_t = dve_const(bias_d[:], [128, NG], f32, "bd")
            ones_t = pconst.tile([128, 1], f32)
            nc.vector.memset(ones_t[:], 1.0)
            ident = pconst.tile([128, 128], bf16)
            nc.sync.dma_start(ident[:], ident_in[:])

            # HT: decoder hidden states, transposed, col = k*SBP + t*64 + b
            ht = pht.tile([128, KC * SBP], bf16)
            nc.vector.memset(ht[:], 0.0)

            def load_w(dram, pool, tag, width=4 * H):
                ts = []
                dw = dram.shape[2]
                for k in range(KC):
                    t = pool.tile([128, width], bf16, tag=f"{tag}{k}")
                    nc.sync.dma_start(t[:, :dw], dram[k])
                    ts.append(t)
                return ts

            we_i = load_w(wi_e, pw, "wie")
            we_h = load_w(wh_e, pw, "whe")
            wd_i = load_w(wi_d, pw, "wid")
            wd_h = load_w(wh_d, pw, "whd")

            # ============ unified 95-step recurrence ============
            with (
                tc.tile_pool(name="psA", bufs=3, space=bass.MemorySpace.PSUM)
                    as psA,
                tc.tile_pool(name="psB", bufs=2, space=bass.MemorySpace.PSUM)
                    as psB,
                tc.tile_pool(name="psC", bufs=2, space=bass.MemorySpace.PSUM)
                    as psC,
            ):
                def bulk_pieces(xt_src, wi_t, bias_t, t0, nsteps):
                    """Yield closures: piece 0 = DMA + gx alloc, one piece
                    per gate chunk (4 MMs + bias copy to gx), then a
                    sentinel returning the gx tile."""
                    w = nsteps * B
                    state = {}

                    def p_dma():
                        state["gx"] = pgx.tile([128, NG * WIN * B], bf16,
                                               tag="gx", name="gxw")
                        xtw = []
                        for k in range(KC):
                            t = pxt.tile([128, WIN * B], bf16, tag=f"xt{k}")
                            nc.sync.dma_start(
                                t[:, :w], xt_src[k * 128:(k + 1) * 128,
                                                 t0 * B:t0 * B + w])
                            xtw.append(t)
                        state["xtw"] = xtw
                    yield p_dma

                    def mk_chunk(g):
                        def p_chunk():
                            pb = psA.tile([128, 512], f32, tag="psA")
                            for k in range(KC):
                                nc.tensor.matmul(
                                    pb[:, :w],
                                    wi_t[k][:, g * 128:(g + 1) * 128],
                                    state["xtw"][k][:, :w],
                                    start=(k == 0), stop=(k == KC - 1))
                            nc.vector.tensor_scalar_add(
                                state["gx"][:, g * WIN * B:g * WIN * B + w],
                                pb[:, :w], bias_t[:, g:g + 1])
                        return p_chunk
                    for g in range(NG):
                        yield mk_chunk(g)
                    yield lambda: state["gx"]

                def lstm_step(gx, lt, h_rhs, c_prev, wh_t, h_out_ap):
                    """One step. g chunks issue first (c-path overlaps the
                    i/f/o matmuls), o last (short h tail)."""
                    pA = psA.tile([128, 512], f32, tag="psA")  # i|f
                    pB = psB.tile([128, 256], f32, tag="psB")  # g
                    pC = psC.tile([128, 256], f32, tag="psC")  # o

                    def dst(c):
                        if c in G_CH:
                            return pB[:, (c - 12) * B:(c - 11) * B]
                        if c in O_CH:
                            return pC[:, (c - 8) * B:(c - 7) * B]
                        return pA[:, c * B:(c + 1) * B]
                    order = G_CH + I_CH + F_CH + O_CH
                    gx_r = gx[:].rearrange("p (g s) -> p g s", g=NG)
                    # x-part injection: identity stationary, one wide
                    # matmul per PSUM tile (one accumulation group each)
                    nc.tensor.matmul(
                        pB[:].rearrange("p (g s) -> p g s", g=4),
                        ident[:], gx_r[:, 12:16, lt * B:(lt + 1) * B],
                        start=True, stop=False)
                    nc.tensor.matmul(
                        pA[:].rearrange("p (g s) -> p g s", g=8),
                        ident[:], gx_r[:, 0:8, lt * B:(lt + 1) * B],
                        start=True, stop=False)
                    nc.tensor.matmul(
                        pC[:].rearrange("p (g s) -> p g s", g=4),
                        ident[:], gx_r[:, 8:12, lt * B:(lt + 1) * B],
                        start=True, stop=False)
                    # h-part; last matmul into each tile carries stop
                    for c in order:
                        for k in range(KC):
                            last = (k == KC - 1) and c in (15, 7, 11)
                            nc.tensor.matmul(
                                dst(c),
                                wh_t[k][:, c * 128:(c + 1) * 128],
                                h_rhs(k),
                                start=False, stop=last)
                    # ACT: g's tanh first (its matmuls finished first)
                    tng = pact.tile([128, 256], f32, tag="tng")
                    nc.scalar.activation(tng[:], pB[:], AF.Tanh)
                    sig = pact.tile([128, 512], f32, tag="sig")
                    nc.scalar.activation(sig[:], pA[:], AF.Sigmoid)
                    sgo = pact.tile([128, 256], f32, tag="sgo")
                    nc.scalar.activation(sgo[:], pC[:], AF.Sigmoid)
                    # c2 = sig_f*c + sig_i*tanh_g
                    t2 = pact.tile([128, 256], f32, tag="t2")
                    nc.vector.tensor_mul(t2[:], sig[:, 0:256], tng[:])
                    t1 = pact.tile([128, 256], f32, tag="t1")
                    nc.vector.tensor_mul(t1[:], sig[:, 256:512], c_prev[:])
                    c_new = pstate.tile([128, 256], f32, tag="c")
                    nc.vector.tensor_add(c_new[:], t1[:], t2[:])
                    tnc = pact.tile([128, 256], f32, tag="tnc")
                    nc.scalar.activation(tnc[:], c_new[:], AF.Tanh)
                    nc.vector.tensor_mul(
                        h_out_ap,
                        sgo[:].rearrange("p (k s) -> p k s", k=KC),
                        tnc[:].rearrange("p (k s) -> p k s", k=KC))
                    return c_new

                h_prev = pstate.tile([128, KC * B], bf16, tag="h")
                nc.vector.memset(h_prev[:], 0.0)
                c_prev = pstate.tile([128, 256], f32, tag="c")
                nc.vector.memset(c_prev[:], 0.0)

                win_list = (
                    [(xt_enc, we_i, bias_e_t, t0, min(WIN, SRC - t0))
                     for t0 in range(0, SRC, WIN)] +
                    [(xt_dec, wd_i, bias_d_t, t0, min(WIN, DEC - t0))
                     for t0 in range(0, DEC, WIN)])

                gx = None
                for p in bulk_pieces(*win_list[0]):   # prologue window
                    r = p()
                    gx = r if r is not None else gx
                next_idx = 1
                next_gen = bulk_pieces(*win_list[next_idx])
                gx_next = None

                step_no = 0
                for phase, nsteps in (("enc", SRC), ("dec", DEC)):
                    wh_t = we_h if phase == "enc" else wd_h
                    for t in range(nsteps):
                        if t % WIN == 0 and step_no > 0:
                            # window switch: finish pending bulk, swap gx
                            while next_gen is not None:
                                try:
                                    p = next(next_gen)
                                except StopIteration:
                                    next_gen = None
                                    break
                                r = p()
                                gx_next = r if r is not None else gx_next
                            gx, gx_next = gx_next, None
                            next_idx += 1
                            if next_idx < len(win_list):
                                next_gen = bulk_pieces(*win_list[next_idx])
                        if phase == "enc" or t == 0:
                            hp = h_prev
                            rhs = (lambda k, hp=hp:
                                   hp[:, k * B:(k + 1) * B])
                        else:
                            rhs = (lambda k, tp=t - 1:
                                   ht[:, k * SBP + tp * B:
                                      k * SBP + (tp + 1) * B])
                        if phase == "enc":
                            h_new = pstate.tile([128, KC * B], bf16, tag="h")
                            out_ap = h_new[:].rearrange(
                                "p (k s) -> p k s", k=KC)
                        else:
                            out_ap = ht[:].rearrange(
                                "p (k s) -> p k s",
                                k=KC)[:, :, t * B:(t + 1) * B]
                        c_new = lstm_step(gx, t % WIN, rhs, c_prev, wh_t,
                                          out_ap)
                        if phase == "enc":
                            mk = pmask.tile([128, KC * B], mybir.dt.uint8,
                                            tag="mk")
                            nc.sync.dma_start(mk[:], mask_in[t])
                            nc.vector.copy_predicated(h_new[:], mk[:],
                                                      h_prev[:])
                            nc.vector.copy_predicated(c_new[:], mk[:],
                                                      c_prev[:])
                            h_prev = h_new
                        c_prev = c_new
                        step_no += 1
                        # interleave next window's bulk (2 pieces/step)
                        if next_gen is not None:
                            for _ in range(2):
                                try:
                                    p = next(next_gen)
                                except StopIteration:
                                    next_gen = None
                                    break
                                r = p()
                                gx_next = r if r is not None else gx_next

            # ================= target logits =================
            with tc.tile_pool(name="psL", bufs=2,
                              space=bass.MemorySpace.PSUM) as psL:
                def load_w2(dram, nm):
                    # two [128, 2*VSP] tiles in the (now idle) gx slots
                    dw = dram.shape[2]
                    ts = []
                    for half in range(2):
                        t = pgx.tile([128, 2 * VSP], bf16, tag="gx",
                                     name=f"{nm}{half}")
                        for j in range(2):
                            nc.sync.dma_start(
                                t[:, j * VSP:j * VSP + dw], dram[half * 2 + j])
                        ts.append(t)
                    return lambda k: ts[k // 2][:, (k % 2) * VSP:
                                                (k % 2 + 1) * VSP]

                wtg_s = load_w2(wtgt_in, "wtg")
                l_sb = pconst.tile([1, SB], f32)
                for nt in range(6):
                    wdt = min(512, SB - nt * 512)
                    pt = psL.tile([128, 2048], f32, tag="psL")
                    for k in range(KC):
                        prod = plog.tile([128, 512], f32, tag="prod")
                        nc.vector.tensor_mul(
                            prod[:, :wdt],
                            ht[:, k * SBP + nt * 512:k * SBP + nt * 512 + wdt],
                            wtg_s(k)[:, nt * 512:nt * 512 + wdt])
                        nc.tensor.matmul(pt[0:1, :wdt], ones_t[:],
                                         prod[:, :wdt],
                                         start=(k == 0), stop=(k == KC - 1))
                    nc.scalar.copy(l_sb[:, nt * 512:nt * 512 + wdt],
                                   pt[0:1, :wdt])
                nc.sync.dma_start(out_l[:], l_sb[:])

                # ============ vocab-shard logits + sum-exp ============
                wot_s = load_w2(wot_in, "wot")
                bout = pconst.tile([128, VSP], bf16)
                nc.sync.dma_start(bout[:], bout_in[:])
                s_all = pconst.tile([128, SBC], f32)

                for sb in range(SBC):
                    sh = []
                    for half in range(2):
                        pl = psL.tile([128, 2048], f32, tag="psL")
                        for v4 in range(4):
                            col = half * 2048 + v4 * 512
                            for k in range(KC):
                                nc.tensor.matmul(
                                    pl[:, v4 * 512:(v4 + 1) * 512],
                                    ht[:, k * SBP + sb * 128:
                                       k * SBP + (sb + 1) * 128],
                                    wot_s(k)[:, col:col + 512],
                                    start=(k == 0), stop=(k == KC - 1))
                        for q in range(2):
                            qs = slice(q * 1024, (q + 1) * 1024)
                            nc.vector.tensor_add(
                                pl[:, qs], pl[:, qs],
                                bout[:, half * 2048 + q * 1024:
                                     half * 2048 + (q + 1) * 1024])
                            sh_t = plog.tile([128, 1], f32,
                                             tag=f"sh{half * 2 + q}",
                                             name="sh_t")
                            nc.scalar.activation(pl[:, qs], pl[:, qs],
                                                 AF.Exp, accum_out=sh_t[:])
                            sh.append(sh_t)
                    sh01 = plog.tile([128, 1], f32, tag="sh01")
                    nc.vector.tensor_add(sh01[:], sh[0][:], sh[1][:])
                    sh23 = plog.tile([128, 1], f32, tag="sh23")
                    nc.vector.tensor_add(sh23[:], sh[2][:], sh[3][:])
                    nc.vector.tensor_add(s_all[:, sb:sb + 1],
                                         sh01[:], sh23[:])
                nc.sync.dma_start(out_s[:], s_all[:])

    nc.compile()
    return nc


def _prep(inputs):
    """Host-side data prep. Returns per-core in_maps + host combine data."""
    il = np.asarray(inputs["input_lines"])
    tl = np.asarray(inputs["target_lines"])
    f = lambda k: np.asarray(inputs[k], np.float32)
    emb_in, emb_tgt = f("emb_in").copy(), f("emb_tgt").copy()
    emb_in[0] = 0.0
    emb_tgt[0] = 0.0
    W_out, b_out = f("W_out"), f("b_out")

    perm = np.concatenate([np.arange(0, 512), np.arange(512, 1024),
                           np.arange(1536, 2048), np.arange(1024, 1536)])

    def wt(w):  # [2048,512] -> [4,128,2048] bf16 (transposed, gate-permuted)
        return np.ascontiguousarray(
            w[perm].T.reshape(KC, 128, 4 * H)).astype(BF16)

    def bias(bi, bh):  # -> [128, 16] f32
        return np.ascontiguousarray(
            (bi + bh)[perm].reshape(NG, 128).T).astype(np.float32)

    x_enc = emb_in[il.reshape(-1)]                       # [3072, 512]
    xt_enc = np.ascontiguousarray(x_enc.T).astype(BF16)  # [512, 3072]
    tgt_in = tl[:DEC].reshape(-1)
    x_dec = emb_tgt[tgt_in]
    xt_dec = np.ascontiguousarray(x_dec.T).astype(BF16)  # [512, 3008]

    m = (il == 0).astype(np.uint8)                       # [48, 64]
    mask = np.ascontiguousarray(np.broadcast_to(
        m[:, None, None, :], (SRC, 128, KC, B)).reshape(
            SRC, 128, KC * B)).astype(np.uint8)

    tgt_next = tl[1:TGT].reshape(-1)                     # [3008]
    wtgt = np.ascontiguousarray(
        W_out[tgt_next].T.reshape(KC, 128, SB)).astype(BF16)
    b_tgt = b_out[tgt_next].astype(np.float64)

    common = dict(
        xt_enc=xt_enc, xt_dec=xt_dec,
        wi_e=wt(f("W_ih_e")), wh_e=wt(f("W_hh_e")),
        wi_d=wt(f("W_ih_d")), wh_d=wt(f("W_hh_d")),
        bias_e=bias(f("b_ih_e"), f("b_hh_e")),
        bias_d=bias(f("b_ih_d"), f("b_hh_d")),
        mask=mask, wtgt=wtgt,
        ident=np.eye(128, dtype=BF16),
    )
    in_maps = []
    for c in range(NCORES):
        ws = np.zeros((VSP, H), np.float32)
        ws[:VSH] = W_out[c * VSH:(c + 1) * VSH]
        bs = np.full(VSP, -88.0, np.float32)
        bs[:VSH] = b_out[c * VSH:(c + 1) * VSH]
        in_maps.append(dict(
            common,
            wot=np.ascontiguousarray(ws.T.reshape(KC, 128, VSP)).astype(BF16),
            bout=np.ascontiguousarray(
                np.broadcast_to(bs, (128, VSP))).astype(BF16),
        ))
    return in_maps, b_tgt


def _combine(results, b_tgt):
    s = np.zeros(SBP, np.float64)
    for r in results:
        s += np.asarray(r["out_s"], np.float64).T.reshape(-1)
    s = s[:SB]
    lse = np.log(s)
    l_tgt = np.asarray(results[0]["out_l"], np.float64).reshape(-1) + b_tgt
    return np.float32((lse - l_tgt).sum() / B)


def kernel(**inputs):
    global _COMPILED
    from concourse.bass_utils import run_bass_kernel_spmd
    in_maps, b_tgt = _prep(inputs)
    if _COMPILED is None:
        _COMPILED = _build()
    res = run_bass_kernel_spmd(_COMPILED, in_maps, list(range(NCORES)))
    return _combine(res.results, b_tgt)


if __name__ == "__main__":
    import reference
    inp = reference.setup_inputs()
    expected = np.asarray(reference.reference(**inp))
    actual = kernel(**{k: np.asarray(v) for k, v in inp.items()})
    err = abs(actual - expected) / max(abs(expected), 1e-9)
    print(f"expected={expected} actual={actual} rel_err={err:.3e}")



# revision 11
# speedup vs baseline: 1.1802x; 1.1802x over previous
"""Encoder-decoder LSTM seq2seq loss kernel for 8 TRN2 NeuronCores.

Strategy (v2):
  - LSTM recurrences (encoder 48 + decoder 47 steps) replicated on every
    core in gate-major layout: gates^T [2048, 64] via 128 [128,64] MMs
    per step over a fused contraction [x_t; h_{t-1}] (1024 = 8 chunks).
    The x-half MMs for step t+1 are issued in step t's tail (no h dep),
    so they fill the PE while the ACT/DVE cell chain runs.
  - Four separate gate PSUM tiles (i, f, o, g), each opened by a K=4
    bias matmul (bias broadcast via indicator rhs) and closed right
    after its own 16 h-MMs, so tanh(g)/sigmoid(i)/sigmoid(f) and the
    c-path all run *during* the burst; only sigmoid(o) -> h remains in
    the tail, chunked in halves so the next burst chases the first half.
  - Decoder logits are computed TRANSPOSED ([vocab_part, step*batch])
    against the core's 4000-row vocab shard in fp8 (DoubleRow, 2x): the
    per-vocab-row bias rides the ACT Exp bias argument, and the softmax
    denominator is reduced over partitions with K=1 ones-matmuls
    accumulating into a [1, 512] PSUM.
  - Target logits l_tgt are a per-sample dot h . W_out[tgt]: h (bf16) is
    DMA'd out and the tiny [3008, 512] dot runs on host along with the
    final log-sum-exp combine.
"""

import sys

sys.path.insert(0, "/opt/trn_rl_repo")

import numpy as np
import ml_dtypes

BF16 = ml_dtypes.bfloat16
FP8 = ml_dtypes.float8_e4m3

# Model dims (hardcoded per contract)
SRC, TGT, B, H, V = 48, 48, 64, 512, 32000
DEC = TGT - 1                  # 47 decoder steps
NSTEP = SRC + DEC              # 95 total steps
SB = DEC * B                   # 3008 (step*batch)
SBP = 3072                     # padded
NCORES = 8
VSH = V // NCORES              # 4000 vocab rows per core
VSP = 4096                     # padded shard
KC = 4                         # hidden chunks (512/128)
WSCALE = 64.0                  # fp8 weight pre-scale
HSCALE = 8.0                   # fp8 hidden-state pre-scale

# gate-chunk indices in the permuted [i f o g] weight layout.
# issue order G, I, F, O (c-path inputs stop early; o last).
GATE_CHUNKS = {"g": [12, 13, 14, 15], "i": [0, 1, 2, 3],
               "f": [4, 5, 6, 7], "o": [8, 9, 10, 11]}
GATE_ORDER = ["g", "i", "f", "o"]

USE_DR = True                  # fp8 DoubleRow for the vocab logits GEMM

_COMPILED = None


def _build():
    import concourse.bass as bass
    import concourse.bacc as bacc
    import concourse.tile as tile
    from concourse import mybir

    f32 = mybir.dt.float32
    bf16 = mybir.dt.bfloat16
    fp8 = mybir.dt.float8e4
    u8 = mybir.dt.uint8
    AF = mybir.ActivationFunctionType
    DR = mybir.MatmulPerfMode.DoubleRow

    nc = bacc.Bacc("TRN2", target_bir_lowering=False, debug=False,
                   num_devices=NCORES)

    def din(name, shape, dt=bf16):
        return nc.dram_tensor(name, shape, dt, kind="ExternalInput").ap()

    xt_e_in = din("xt_e", [KC, 128, SRC * B])
    xt_d_in = din("xt_d", [KC, 128, DEC * B])
    wi_e_in = din("wi_e", [KC, 128, 4 * H])
    wh_e_in = din("wh_e", [KC, 128, 4 * H])
    wi_d_in = din("wi_d", [KC, 128, 4 * H])
    wh_d_in = din("wh_d", [KC, 128, 4 * H])
    bias_e_in = din("bias_e", [4, 512])
    bias_d_in = din("bias_d", [4, 512])
    ind_in = din("ind", [4, 256])
    mask_in = din("mask", [SRC, 128, KC * B], u8)
    wot_in = din("wot8", [128, KC, VSP], fp8)
    bexp_in = din("bexp", [128, VSP // 128], f32)

    out_s = nc.dram_tensor("out_s", [1, SBP], f32, kind="ExternalOutput").ap()
    out_h = nc.dram_tensor("out_h", [128, KC * SBP], bf16,
                           kind="ExternalOutput").ap()

    with tile.TileContext(nc) as tc:
        from contextlib import ExitStack
        with ExitStack() as ctx:
            # ---- pools ----
            pconst = ctx.enter_context(tc.tile_pool(name="const", bufs=1))
            pw = ctx.enter_context(tc.tile_pool(name="w", bufs=1))
            pxt = ctx.enter_context(tc.tile_pool(name="xt", bufs=1))
            pht = ctx.enter_context(tc.tile_pool(name="ht", bufs=1))
            pstate = ctx.enter_context(tc.tile_pool(name="state", bufs=3))
            pact = ctx.enter_context(tc.tile_pool(name="act", bufs=1))
            pmask = ctx.enter_context(tc.tile_pool(name="mask", bufs=2))
            pexp = ctx.enter_context(tc.tile_pool(name="exp", bufs=3))

            # ---- constants / weights (DMA order = need order) ----
            bias_e_t = pconst.tile([4, 512], bf16)
            nc.sync.dma_start(bias_e_t[:], bias_e_in[:])
            ind_t = pconst.tile([4, 256], bf16)
            nc.sync.dma_start(ind_t[:], ind_in[:])

            # encoder x^T: head (steps 0-7) in separate tiles so the
            # prologue doesn't wait on the full 3MB transfer
            XHEAD = 8
            xt_e_h, xt_e_t = [], []
            for k in range(KC):
                t = pxt.tile([128, XHEAD * B], bf16, tag=f"xteh{k}")
                nc.sync.dma_start(t[:], xt_e_in[k, :, :XHEAD * B])
                xt_e_h.append(t)

            def load_w(dram, tag):
                ts = []
                for k in range(KC):
                    t = pw.tile([128, 4 * H], bf16, tag=f"{tag}{k}")
                    nc.sync.dma_start(t[:], dram[k])
                    ts.append(t)
                return ts

            wi_e = load_w(wi_e_in, "wie")
            wh_e = load_w(wh_e_in, "whe")
            for k in range(KC):
                t = pxt.tile([128, (SRC - XHEAD) * B], bf16, tag=f"xtet{k}")
                nc.sync.dma_start(t[:], xt_e_in[k, :, XHEAD * B:])
                xt_e_t.append(t)
            bias_d_t = pconst.tile([4, 512], bf16)
            nc.sync.dma_start(bias_d_t[:], bias_d_in[:])
            xt_d = []
            for k in range(KC):
                t = pxt.tile([128, SBP], bf16, tag=f"xtd{k}")
                nc.sync.dma_start(t[:, :DEC * B], xt_d_in[k])
                xt_d.append(t)
            wi_d = load_w(wi_d_in, "wid")
            wh_d = load_w(wh_d_in, "whd")

            wot8 = pconst.tile([128, KC, VSP], fp8)
            nc.sync.dma_start(wot8[:], wot_in[:])
            bexp_t = pconst.tile([128, VSP // 128], f32)
            nc.sync.dma_start(bexp_t[:], bexp_in[:])

            ones_t = pconst.tile([128, 1], bf16)
            nc.vector.memset(ones_t[:], 1.0)

            # decoder hidden states, transposed: [128, k, t*64+b]
            ht = pht.tile([128, KC, SBP], bf16)

            # ============ recurrence ============
            with (
                tc.tile_pool(name="psG", bufs=1, space=bass.MemorySpace.PSUM)
                    as psG,
                tc.tile_pool(name="psI", bufs=1, space=bass.MemorySpace.PSUM)
                    as psI,
                tc.tile_pool(name="psF", bufs=1, space=bass.MemorySpace.PSUM)
                    as psF,
                tc.tile_pool(name="psO", bufs=1, space=bass.MemorySpace.PSUM)
                    as psO,
            ):
                pools = {"g": psG, "i": psI, "f": psF, "o": psO}

                def xsel_e(k, t):
                    if t < XHEAD:
                        return xt_e_h[k][:, t * B:(t + 1) * B]
                    return xt_e_t[k][:, (t - XHEAD) * B:(t - XHEAD + 1) * B]

                def xsel_d(k, t):
                    return xt_d[k][:, t * B:(t + 1) * B]

                # steps: (wi, wh, xsel, bias, phase)
                steps = ([(wi_e, wh_e, xsel_e, bias_e_t, "enc")] * SRC +
                         [(wi_d, wh_d, xsel_d, bias_d_t, "dec")] * DEC)

                def x_block(s, gtiles):
                    """bias-MM (start) + 16 x-part MMs per gate for step s.
                    Allocates the four gate psum tiles for step s."""
                    wi, _, xsel, bias_t, ph = steps[s]
                    t = s if ph == "enc" else s - SRC
                    for gi, gname in enumerate(GATE_ORDER):
                        pt = pools[gname].tile([128, 256], f32,
                                               padded_shape=[128, 512],
                                               tag=gname, name=f"p_{gname}")
                        gtiles[gname] = pt
                        # gate index in the permuted layout (i,f,o,g blocks)
                        gt = {"i": 0, "f": 1, "o": 2, "g": 3}[gname]
                        nc.tensor.matmul(
                            pt[:], bias_t[0:4, gt * 128:(gt + 1) * 128],
                            ind_t[0:4, :], start=True, stop=False)
                        for ci, c in enumerate(GATE_CHUNKS[gname]):
                            for k in range(KC):
                                nc.tensor.matmul(
                                    pt[:, ci * 64:(ci + 1) * 64],
                                    wi[k][:, c * 128:(c + 1) * 128],
                                    xsel(k, t), start=False, stop=False)

                def h_mms(gname, pt, wh, h_rhs):
                    for ci, c in enumerate(GATE_CHUNKS[gname]):
                        for k in range(KC):
                            last = (ci == 3 and k == KC - 1)
                            nc.tensor.matmul(
                                pt[:, ci * 64:(ci + 1) * 64],
                                wh[k][:, c * 128:(c + 1) * 128],
                                h_rhs(k), start=False, stop=last)

                h_prev = pstate.tile([128, KC * B], bf16, tag="h")
                nc.vector.memset(h_prev[:], 0.0)
                c_prev = pstate.tile([128, 256], f32, tag="c")
                nc.vector.memset(c_prev[:], 0.0)

                gtiles = {}
                x_block(0, gtiles)          # prologue

                for s in range(NSTEP):
                    _, wh, _, _, ph = steps[s]
                    t = s if ph == "enc" else s - SRC
                    if ph == "enc" or t == 0:
                        hp = h_prev
                        rhs = (lambda k, hp=hp: hp[:, k * B:(k + 1) * B])
                    else:
                        rhs = (lambda k, tp=t - 1:
                               ht[:, k, tp * B:(tp + 1) * B])

                    if ph == "enc":
                        mk = pmask.tile([128, KC * B], u8, tag="mk")
                        nc.sync.dma_start(mk[:], mask_in[s])

                    pG, pI = gtiles["g"], gtiles["i"]
                    pF, pO = gtiles["f"], gtiles["o"]

                    # -------- burst: h-MMs with per-gate early stops ----
                    h_mms("g", pG, wh, rhs)
                    tng = pact.tile([128, 256], f32, tag="tng")
                    nc.scalar.activation(tng[:], pG[:], AF.Tanh)
                    h_mms("i", pI, wh, rhs)
                    sgi = pact.tile([128, 256], f32, tag="sgi")
                    nc.scalar.activation(sgi[:], pI[:], AF.Sigmoid)
                    t2 = pact.tile([128, 256], f32, tag="t2")
                    nc.vector.tensor_mul(t2[:], sgi[:], tng[:])
                    h_mms("f", pF, wh, rhs)
                    sgf = pact.tile([128, 256], f32, tag="sgf")
                    nc.scalar.activation(sgf[:], pF[:], AF.Sigmoid)
                    t1 = pact.tile([128, 256], f32, tag="t1")
                    nc.vector.tensor_mul(t1[:], sgf[:], c_prev[:])
                    c_new = pstate.tile([128, 256], f32, tag="c")
                    nc.vector.tensor_add(c_new[:], t1[:], t2[:])
                    h_mms("o", pO, wh, rhs)
                    sgo = pact.tile([128, 256], f32, tag="sgo")
                    tnc = pact.tile([128, 256], f32, tag="tnc")

                    if ph == "enc":
                        h_new = pstate.tile([128, KC * B], bf16, tag="h")
                        out_full = h_new[:].rearrange("p (k s) -> p k s", k=KC)
                    else:
                        out_full = ht[:, :, t * B:(t + 1) * B]

                    # o-tail in halves so the next burst starts on half 0
                    for hh in range(2):
                        cs = slice(hh * 128, (hh + 1) * 128)
                        ks = slice(hh * 2, hh * 2 + 2)
                        nc.scalar.activation(sgo[:, cs], pO[:, cs], AF.Sigmoid)
                        nc.scalar.activation(tnc[:, cs], c_new[:, cs], AF.Tanh)
                        nc.vector.tensor_mul(
                            out_full[:, ks, :],
                            sgo[:, cs].rearrange("p (k s) -> p k s", k=2),
                            tnc[:, cs].rearrange("p (k s) -> p k s", k=2))
                        if ph == "enc":
                            nc.vector.copy_predicated(
                                h_new[:, cs], mk[:, cs], h_prev[:, cs])
                    if ph == "enc":
                        nc.vector.copy_predicated(c_new[:], mk[:], c_prev[:])
                        h_prev = h_new
                    c_prev = c_new

                    # -------- tail filler: next step's bias + x MMs ------
                    gtiles = {}
                    if s + 1 < NSTEP:
                        x_block(s + 1, gtiles)

            # ============ transition ============
            nc.sync.dma_start(out_h[:], ht[:].rearrange("p k s -> p (k s)"))
            # fp8 cast of decoder h (x HSCALE) for the DR logits GEMM
            ht8 = []
            for half in range(2):
                t8 = pconst.tile([128, 2, SBP], fp8, name=f"ht8_{half}")
                nc.scalar.mul(
                    t8[:].rearrange("p k s -> p (k s)"),
                    ht[:, half * 2:half * 2 + 2, :].rearrange(
                        "p k s -> p (k s)"), HSCALE)
                ht8.append(t8)

            # ============ vocab-shard logits + sum-exp ============
            s_sb = pconst.tile([1, SBP], f32)
            NV = VSP // 128            # 32 vocab chunks per core
            with (
                tc.tile_pool(name="psL", bufs=3, space=bass.MemorySpace.PSUM)
                    as psL,
                tc.tile_pool(name="psA", bufs=2, space=bass.MemorySpace.PSUM)
                    as psA,
            ):
                for grp in range((SB + 511) // 512):
                    w = min(512, SB - grp * 512)
                    col = slice(grp * 512, grp * 512 + w)
                    acc = psA.tile([1, 512], f32, tag="acc")
                    for v in range(NV):
                        pv = psL.tile([128, 512], f32, tag="pv")
                        if USE_DR:
                            for kp in range(2):
                                nc.tensor.matmul(
                                    pv[:, :w],
                                    wot8[:, kp * 2:kp * 2 + 2,
                                         v * 128:(v + 1) * 128],
                                    ht8[kp][:, :, col],
                                    start=(kp == 0), stop=(kp == 1),
                                    perf_mode=DR)
                        else:
                            for k in range(KC):
                                nc.tensor.matmul(
                                    pv[:, :w],
                                    wot8[:, k, v * 128:(v + 1) * 128],
                                    ht8[k // 2][:, k % 2, col],
                                    start=(k == 0), stop=(k == KC - 1))
                        ex = pexp.tile([128, 512], bf16, tag="ex")
                        nc.scalar.activation(ex[:, :w], pv[:, :w], AF.Exp,
                                             bias=bexp_t[:, v:v + 1],
                                             scale=1.0 / (WSCALE * HSCALE))
                        nc.tensor.matmul(acc[0:1, :w], ones_t[:, 0:1],
                                         ex[:, :w],
                                         start=(v == 0), stop=(v == NV - 1))
                    nc.scalar.copy(s_sb[0:1, col], acc[0:1, :w])
            nc.sync.dma_start(out_s[:], s_sb[:])

    nc.compile()
    return nc


def _prep(inputs):
    """Host-side data prep. Returns per-core in_maps + host combine data."""
    il = np.asarray(inputs["input_lines"])
    tl = np.asarray(inputs["target_lines"])
    f = lambda k: np.asarray(inputs[k], np.float32)
    emb_in, emb_tgt = f("emb_in").copy(), f("emb_tgt").copy()
    emb_in[0] = 0.0
    emb_tgt[0] = 0.0
    W_out, b_out = f("W_out"), f("b_out")

    perm = np.concatenate([np.arange(0, 512), np.arange(512, 1024),
                           np.arange(1536, 2048), np.arange(1024, 1536)])

    def wt(w):  # [2048,512] -> [4,128,2048] bf16 (transposed, gate-permuted)
        return np.ascontiguousarray(
            w[perm].T.reshape(KC, 128, 4 * H)).astype(BF16)

    def bias(bi, bh):  # -> [4, 512] bf16 lhsT: [k, gt*128+p]
        bfull = (bi + bh)[perm].reshape(4, 4, 128)      # [gt, k, p]
        return np.ascontiguousarray(
            bfull.transpose(1, 0, 2).reshape(4, 512)).astype(BF16)

    def xt(emb, toks):  # -> [4, 128, T*B] bf16
        x = emb[toks.reshape(-1)]                       # [T*B, 512]
        return np.ascontiguousarray(
            x.T.reshape(KC, 128, -1)).astype(BF16)

    m = (il == 0).astype(np.uint8)                       # [48, 64]
    mask = np.ascontiguousarray(np.broadcast_to(
        m[:, None, None, :], (SRC, 128, KC, B)).reshape(
            SRC, 128, KC * B)).astype(np.uint8)

    ind = np.zeros((4, 256), BF16)
    for k in range(4):
        ind[k, k * 64:(k + 1) * 64] = 1.0

    common = dict(
        xt_e=xt(emb_in, il), xt_d=xt(emb_tgt, tl[:DEC]),
        wi_e=wt(f("W_ih_e")), wh_e=wt(f("W_hh_e")),
        wi_d=wt(f("W_ih_d")), wh_d=wt(f("W_hh_d")),
        bias_e=bias(f("b_ih_e"), f("b_hh_e")),
        bias_d=bias(f("b_ih_d"), f("b_hh_d")),
        mask=mask, ind=ind,
    )
    in_maps = []
    for c in range(NCORES):
        ws = np.zeros((VSP, H), np.float32)
        ws[:VSH] = W_out[c * VSH:(c + 1) * VSH] * WSCALE
        wot8 = np.ascontiguousarray(
            ws.T.reshape(KC, 128, VSP).transpose(1, 0, 2)).astype(FP8)
        bx = np.full(VSP, -88.0, np.float32)
        bx[:VSH] = b_out[c * VSH:(c + 1) * VSH]
        bexp = np.ascontiguousarray(
            bx.reshape(VSP // 128, 128).T).astype(np.float32)
        in_maps.append(dict(common, wot8=wot8, bexp=bexp))

    tgt_next = tl[1:TGT].reshape(-1)                     # [3008]
    w_tgt = W_out[tgt_next]                              # [3008, 512]
    b_tgt = b_out[tgt_next].astype(np.float64)
    return in_maps, (w_tgt, b_tgt)


def _combine(results, tgt_data):
    w_tgt, b_tgt = tgt_data
    s = np.zeros(SBP, np.float64)
    for r in results:
        s += np.asarray(r["out_s"], np.float64).reshape(-1)
    lse = np.log(s[:SB])
    # l_tgt = h . W_out[tgt] + b[tgt] from the DMA'd decoder h (core 0)
    hT = np.asarray(results[0]["out_h"], np.float32).reshape(128, KC, SBP)
    h = hT[:, :, :SB].transpose(2, 1, 0).reshape(SB, H)  # [t*B, k*128+p]
    l_tgt = np.einsum("ij,ij->i", h, w_tgt.astype(np.float32),
                      dtype=np.float64) + b_tgt
    return np.float32((lse - l_tgt).sum() / B)


def kernel(**inputs):
    global _COMPILED
    from concourse.bass_utils import run_bass_kernel_spmd
    in_maps, tgt_data = _prep(inputs)
    if _COMPILED is None:
        _COMPILED = _build()
    res = run_bass_kernel_spmd(_COMPILED, in_maps, list(range(NCORES)))
    return _combine(res.results, tgt_data)


if __name__ == "__main__":
    import reference
    inp = reference.setup_inputs()
    expected = np.asarray(reference.reference(**inp))
    actual = kernel(**{k: np.asarray(v) for k, v in inp.items()})
    err = abs(actual - expected) / max(abs(expected), 1e-9)
    print(f"expected={expected} actual={actual} rel_err={err:.3e}")


# revision 26
# speedup vs baseline: 1.4051x; 1.1905x over previous
"""Encoder-decoder LSTM seq2seq loss kernel for 8 TRN2 NeuronCores.

Strategy (v2):
  - LSTM recurrences (encoder 48 + decoder 47 steps) replicated on every
    core in gate-major layout: gates^T [2048, 64] via 128 [128,64] MMs
    per step over a fused contraction [x_t; h_{t-1}] (1024 = 8 chunks).
    The x-half MMs for step t+1 are issued in step t's tail (no h dep),
    so they fill the PE while the ACT/DVE cell chain runs.
  - Four separate gate PSUM tiles (i, f, o, g), each opened by a K=4
    bias matmul (bias broadcast via indicator rhs) and closed right
    after its own 16 h-MMs, so tanh(g)/sigmoid(i)/sigmoid(f) and the
    c-path all run *during* the burst; only sigmoid(o) -> h remains in
    the tail, chunked in halves so the next burst chases the first half.
  - Decoder logits are computed TRANSPOSED ([vocab_part, step*batch])
    against the core's 4000-row vocab shard in fp8 (DoubleRow, 2x): the
    per-vocab-row bias rides the ACT Exp bias argument, and the softmax
    denominator is reduced over partitions with K=1 ones-matmuls
    accumulating into a [1, 512] PSUM.
  - Target logits l_tgt are a per-sample dot h . W_out[tgt]: h (bf16) is
    DMA'd out and the tiny [3008, 512] dot runs on host along with the
    final log-sum-exp combine.
"""

import sys

sys.path.insert(0, "/opt/trn_rl_repo")

import numpy as np
import ml_dtypes

BF16 = ml_dtypes.bfloat16
FP8 = ml_dtypes.float8_e4m3

# Model dims (hardcoded per contract)
SRC, TGT, B, H, V = 48, 48, 64, 512, 32000
DEC = TGT - 1                  # 47 decoder steps
NSTEP = SRC + DEC              # 95 total steps
SB = DEC * B                   # 3008 (step*batch)
SBP = 3072                     # padded
NCORES = 8
VSH = V // NCORES              # 4000 vocab rows per core
VSP = 4096                     # padded shard
KC = 4                         # hidden chunks (512/128)
WSCALE = 64.0                  # fp8 weight pre-scale
HSCALE = 8.0                   # fp8 hidden-state pre-scale

# gate-chunk indices in the permuted [i f o g] weight layout.
# issue order G, I, F, O (c-path inputs stop early; o last).
GATE_CHUNKS = {"g": [12, 13, 14, 15], "i": [0, 1, 2, 3],
               "f": [4, 5, 6, 7], "o": [8, 9, 10, 11]}
GATE_ORDER = ["g", "i", "f", "o"]

USE_DR = True                  # fp8 DoubleRow for the vocab logits GEMM

_COMPILED = None


def _build():
    import concourse.bass as bass
    import concourse.bacc as bacc
    import concourse.tile as tile
    from concourse import mybir

    f32 = mybir.dt.float32
    bf16 = mybir.dt.bfloat16
    fp8 = mybir.dt.float8e4
    u8 = mybir.dt.uint8
    AF = mybir.ActivationFunctionType
    DR = mybir.MatmulPerfMode.DoubleRow

    nc = bacc.Bacc("TRN2", target_bir_lowering=False, debug=False,
                   num_devices=NCORES)

    def din(name, shape, dt=bf16):
        return nc.dram_tensor(name, shape, dt, kind="ExternalInput").ap()

    xt_e_in = din("xt_e", [KC, 128, SRC * B])
    xt_d_in = din("xt_d", [KC, 128, DEC * B])
    wi_e_in = din("wi_e", [KC, 128, 4 * H])
    wh_e_in = din("wh_e", [KC, 128, 4 * H])
    wi_d_in = din("wi_d", [KC, 128, 4 * H])
    wh_d_in = din("wh_d", [KC, 128, 4 * H])
    bias_e_in = din("bias_e", [128, 512])
    bias_d_in = din("bias_d", [128, 512])
    ind_in = din("ind", [128, 256])
    mask_in = din("mask", [128, SRC * KC * B], u8)
    wot_in = din("wot8", [128, KC, VSP], fp8)
    bout_in = din("bout", [128, VSP])

    SBC = (SB + 127) // 128        # 24 sample chunks
    out_s = nc.dram_tensor("out_s", [128, SBC], f32,
                           kind="ExternalOutput").ap()
    out_h = nc.dram_tensor("out_h", [128, KC * SBP], bf16,
                           kind="ExternalOutput").ap()

    with tile.TileContext(nc) as tc:
        from contextlib import ExitStack
        with ExitStack() as ctx:
            # ---- pools ----
            pconst = ctx.enter_context(tc.tile_pool(name="const", bufs=1))
            pw = ctx.enter_context(tc.tile_pool(name="w", bufs=1))
            pxt = ctx.enter_context(tc.tile_pool(name="xt", bufs=1))
            pht = ctx.enter_context(tc.tile_pool(name="ht", bufs=1))
            pstate = ctx.enter_context(tc.tile_pool(name="state", bufs=3))
            pact = ctx.enter_context(tc.tile_pool(name="act", bufs=1))
            pexp = ctx.enter_context(tc.tile_pool(name="exp", bufs=3))

            # ---- constants / weights (DMA order = need order) ----
            bias_e_t = pconst.tile([128, 512], bf16)
            nc.sync.dma_start(bias_e_t[:], bias_e_in[:])
            ind_t = pconst.tile([128, 256], bf16)
            nc.sync.dma_start(ind_t[:], ind_in[:])

            # encoder x^T: head (steps 0-7) in separate tiles so the
            # prologue doesn't wait on the full 3MB transfer
            XHEAD = 8
            xt_e_h, xt_e_t = [], []
            for k in range(KC):
                t = pxt.tile([128, XHEAD * B], bf16, tag=f"xteh{k}")
                nc.sync.dma_start(t[:], xt_e_in[k, :, :XHEAD * B])
                xt_e_h.append(t)

            def load_w(dram, tag):
                ts = []
                for k in range(KC):
                    t = pw.tile([128, 4 * H], bf16, tag=f"{tag}{k}")
                    nc.sync.dma_start(t[:], dram[k])
                    ts.append(t)
                return ts

            wi_e = load_w(wi_e_in, "wie")
            wh_e = load_w(wh_e_in, "whe")
            # all 48 encoder masks in one tile, before the decoder DMAs
            mask_t = pconst.tile([128, SRC * KC * B], u8)
            nc.sync.dma_start(mask_t[:, :8 * KC * B],
                              mask_in[:, :8 * KC * B])
            nc.sync.dma_start(mask_t[:, 8 * KC * B:],
                              mask_in[:, 8 * KC * B:])
            for k in range(KC):
                t = pxt.tile([128, (SRC - XHEAD) * B], bf16, tag=f"xtet{k}")
                nc.sync.dma_start(t[:], xt_e_in[k, :, XHEAD * B:])
                xt_e_t.append(t)
            bias_d_t = pconst.tile([128, 512], bf16)
            nc.sync.dma_start(bias_d_t[:], bias_d_in[:])
            xt_d = []
            for k in range(KC):
                t = pxt.tile([128, SBP], bf16, tag=f"xtd{k}")
                nc.sync.dma_start(t[:, :DEC * B], xt_d_in[k])
                xt_d.append(t)
            wi_d = load_w(wi_d_in, "wid")
            wh_d = load_w(wh_d_in, "whd")

            wot8 = pconst.tile([128, KC, VSP], fp8)
            nc.sync.dma_start(wot8[:], wot_in[:])
            bout_t = pconst.tile([128, VSP], bf16)
            nc.sync.dma_start(bout_t[:], bout_in[:])

            # decoder hidden states, transposed: [128, k, t*64+b]
            ht = pht.tile([128, KC, SBP], bf16)
            nc.vector.memset(ht[:, :, DEC * B:], 0.0)
            # fp8 copy (x HSCALE), filled incrementally during the decoder
            ht8 = [pconst.tile([128, 2, SBP], fp8, name=f"ht8_{i}")
                   for i in range(2)]

            # ============ recurrence ============
            with (
                tc.tile_pool(name="psG", bufs=1, space=bass.MemorySpace.PSUM)
                    as psG,
                tc.tile_pool(name="psI", bufs=1, space=bass.MemorySpace.PSUM)
                    as psI,
                tc.tile_pool(name="psF", bufs=1, space=bass.MemorySpace.PSUM)
                    as psF,
                tc.tile_pool(name="psO", bufs=2, space=bass.MemorySpace.PSUM)
                    as psO,
            ):
                pools = {"g": psG, "i": psI, "f": psF, "o": psO}

                def xsel_e(k, t):
                    if t < XHEAD:
                        return xt_e_h[k][:, t * B:(t + 1) * B]
                    return xt_e_t[k][:, (t - XHEAD) * B:(t - XHEAD + 1) * B]

                def xsel_d(k, t):
                    return xt_d[k][:, t * B:(t + 1) * B]

                # steps: (wi, wh, xsel, bias, phase)
                steps = ([(wi_e, wh_e, xsel_e, bias_e_t, "enc")] * SRC +
                         [(wi_d, wh_d, xsel_d, bias_d_t, "dec")] * DEC)

                def x_block(s, gtiles):
                    """bias-MM (start) + 16 x-part MMs per gate for step s.
                    Allocates the four gate psum tiles for step s."""
                    wi, _, xsel, bias_t, ph = steps[s]
                    t = s if ph == "enc" else s - SRC
                    for gi, gname in enumerate(GATE_ORDER):
                        pt = pools[gname].tile([128, 256], f32,
                                               padded_shape=[128, 512],
                                               tag=gname, name=f"p_{gname}")
                        gtiles[gname] = pt
                        # gate index in the permuted layout (i,f,o,g blocks)
                        gt = {"i": 0, "f": 1, "o": 2, "g": 3}[gname]
                        nc.tensor.matmul(
                            pt[:], bias_t[:, gt * 128:(gt + 1) * 128],
                            ind_t[:], start=True, stop=False)
                        for ci, c in enumerate(GATE_CHUNKS[gname]):
                            for k in range(KC):
                                nc.tensor.matmul(
                                    pt[:, ci * 64:(ci + 1) * 64],
                                    wi[k][:, c * 128:(c + 1) * 128],
                                    xsel(k, t), start=False, stop=False)

                def h_mms(gname, pt, wh, h_rhs):
                    for ci, c in enumerate(GATE_CHUNKS[gname]):
                        for k in range(KC):
                            last = (ci == 3 and k == KC - 1)
                            nc.tensor.matmul(
                                pt[:, ci * 64:(ci + 1) * 64],
                                wh[k][:, c * 128:(c + 1) * 128],
                                h_rhs(k), start=False, stop=last)

                h_prev = pstate.tile([128, KC * B], bf16, tag="h")
                nc.vector.memset(h_prev[:], 0.0)
                c_prev = pstate.tile([128, 256], f32, tag="c")
                nc.vector.memset(c_prev[:], 0.0)

                gtiles = {}
                x_block(0, gtiles)          # prologue

                for s in range(NSTEP):
                    _, wh, _, _, ph = steps[s]
                    t = s if ph == "enc" else s - SRC
                    if ph == "enc" or t == 0:
                        hp = h_prev
                        rhs = (lambda k, hp=hp: hp[:, k * B:(k + 1) * B])
                    else:
                        rhs = (lambda k, tp=t - 1:
                               ht[:, k, tp * B:(tp + 1) * B])

                    if ph == "enc":
                        mk = mask_t[:, s * KC * B:(s + 1) * KC * B]

                    pG, pI = gtiles["g"], gtiles["i"]
                    pF, pO = gtiles["f"], gtiles["o"]

                    # -------- burst: h-MMs with per-gate early stops ----
                    h_mms("g", pG, wh, rhs)
                    tng = pact.tile([128, 256], f32, tag="tng")
                    nc.scalar.activation(tng[:], pG[:], AF.Tanh)
                    h_mms("i", pI, wh, rhs)
                    sgi = pact.tile([128, 256], f32, tag="sgi")
                    nc.scalar.activation(sgi[:], pI[:], AF.Sigmoid)
                    t2 = pact.tile([128, 256], f32, tag="t2")
                    nc.vector.tensor_mul(t2[:], sgi[:], tng[:])
                    h_mms("f", pF, wh, rhs)
                    sgf = pact.tile([128, 256], f32, tag="sgf")
                    nc.scalar.activation(sgf[:], pF[:], AF.Sigmoid)
                    t1 = pact.tile([128, 256], f32, tag="t1")
                    nc.vector.tensor_mul(t1[:], sgf[:], c_prev[:])
                    c_new = pstate.tile([128, 256], f32, tag="c")
                    nc.vector.tensor_add(c_new[:], t1[:], t2[:])
                    h_mms("o", pO, wh, rhs)
                    sgo = pact.tile([128, 256], f32, tag="sgo")
                    tnc = pact.tile([128, 256], f32, tag="tnc")

                    if ph == "enc":
                        h_new = pstate.tile([128, KC * B], bf16, tag="h")
                        out_full = h_new[:].rearrange("p (k s) -> p k s", k=KC)
                    else:
                        out_full = ht[:, :, t * B:(t + 1) * B]

                    # o-tail in halves so the next burst starts on half 0
                    for hh in range(2):
                        cs = slice(hh * 128, (hh + 1) * 128)
                        ks = slice(hh * 2, hh * 2 + 2)
                        nc.scalar.activation(sgo[:, cs], pO[:, cs], AF.Sigmoid)
                        nc.scalar.activation(tnc[:, cs], c_new[:, cs], AF.Tanh)
                        nc.vector.tensor_mul(
                            out_full[:, ks, :],
                            sgo[:, cs].rearrange("p (k s) -> p k s", k=2),
                            tnc[:, cs].rearrange("p (k s) -> p k s", k=2))
                        if ph == "enc":
                            nc.vector.copy_predicated(
                                h_new[:, cs], mk[:, cs], h_prev[:, cs])
                        else:
                            nc.vector.tensor_scalar_mul(
                                ht8[hh][:, :, t * B:(t + 1) * B],
                                out_full[:, ks, :], HSCALE)
                    if ph == "enc":
                        nc.vector.copy_predicated(c_new[:], mk[:], c_prev[:])
                        h_prev = h_new
                    c_prev = c_new

                    # -------- tail filler: next step's bias + x MMs ------
                    gtiles = {}
                    if s + 1 < NSTEP:
                        x_block(s + 1, gtiles)

            # ============ transition ============
            nc.sync.dma_start(out_h[:], ht[:].rearrange("p k s -> p (k s)"))

            # ==== vocab-shard logits + sum-exp (sample-major, fp8 DR) ====
            # per sample-chunk sb: psum [128 samples, 1024 vocab] pairs;
            # bias added by DVE into psum; Exp in-place with accum_out.
            s_all = pconst.tile([128, SBC], f32)
            nc.vector.memset(s_all[:], 0.0)
            esc = 1.0 / (WSCALE * HSCALE)
            with tc.tile_pool(name="psL", bufs=3,
                              space=bass.MemorySpace.PSUM) as psL:
                for sb in range(SBC):
                    wp = min(128, SB - sb * 128)     # samples this chunk
                    scol = slice(sb * 128, sb * 128 + wp)
                    shs = []
                    for pp in range(VSP // 1024):    # 4 vocab pairs
                        pv = psL.tile([128, 1024], f32, tag="pv")
                        for vg in (2 * pp, 2 * pp + 1):
                            half = slice((vg % 2) * 512, (vg % 2 + 1) * 512)
                            for kp in range(2):
                                nc.tensor.matmul(
                                    pv[0:wp, half],
                                    ht8[kp][:, :, scol],
                                    wot8[:, kp * 2:kp * 2 + 2,
                                         vg * 512:(vg + 1) * 512],
                                    start=(kp == 0), stop=(kp == 1),
                                    perf_mode=DR)
                        nc.vector.tensor_add(
                            pv[0:wp, :], pv[0:wp, :],
                            bout_t[0:wp, pp * 1024:(pp + 1) * 1024])
                        sh = pexp.tile([128, 1], f32, tag=f"sh{pp}",
                                       name="sh")
                        nc.scalar.activation(pv[0:wp, :], pv[0:wp, :],
                                             AF.Exp, scale=esc,
                                             accum_out=sh[0:wp, :])
                        shs.append(sh)
                    s01 = pexp.tile([128, 1], f32, tag="s01")
                    nc.vector.tensor_add(s01[0:wp], shs[0][0:wp],
                                         shs[1][0:wp])
                    s23 = pexp.tile([128, 1], f32, tag="s23")
                    nc.vector.tensor_add(s23[0:wp], shs[2][0:wp],
                                         shs[3][0:wp])
                    nc.vector.tensor_add(s_all[0:wp, sb:sb + 1],
                                         s01[0:wp], s23[0:wp])
            nc.sync.dma_start(out_s[:], s_all[:])

    nc.compile()
    return nc


def _prep(inputs):
    """Host-side data prep. Returns per-core in_maps + host combine data."""
    il = np.asarray(inputs["input_lines"])
    tl = np.asarray(inputs["target_lines"])
    f = lambda k: np.asarray(inputs[k], np.float32)
    emb_in, emb_tgt = f("emb_in").copy(), f("emb_tgt").copy()
    emb_in[0] = 0.0
    emb_tgt[0] = 0.0
    W_out, b_out = f("W_out"), f("b_out")

    perm = np.concatenate([np.arange(0, 512), np.arange(512, 1024),
                           np.arange(1536, 2048), np.arange(1024, 1536)])

    def wt(w):  # [2048,512] -> [4,128,2048] bf16 (transposed, gate-permuted)
        return np.ascontiguousarray(
            w[perm].T.reshape(KC, 128, 4 * H)).astype(BF16)

    def bias(bi, bh):  # -> [128, 512] bf16 lhsT (rows 0-3): [k, gt*128+p]
        bfull = (bi + bh)[perm].reshape(4, 4, 128)      # [gt, k, p]
        out = np.zeros((128, 512), np.float32)
        out[:4] = bfull.transpose(1, 0, 2).reshape(4, 512)
        return out.astype(BF16)

    def xt(emb, toks):  # -> [4, 128, T*B] bf16
        x = emb[toks.reshape(-1)]                       # [T*B, 512]
        return np.ascontiguousarray(
            x.T.reshape(KC, 128, -1)).astype(BF16)

    m = (il == 0).astype(np.uint8)                       # [48, 64]
    mask = np.ascontiguousarray(np.broadcast_to(
        m[:, None, None, :], (SRC, 128, KC, B)).transpose(1, 0, 2, 3)
        .reshape(128, SRC * KC * B)).astype(np.uint8)

    ind = np.zeros((128, 256), BF16)
    for k in range(4):
        ind[k, k * 64:(k + 1) * 64] = 1.0

    common = dict(
        xt_e=xt(emb_in, il), xt_d=xt(emb_tgt, tl[:DEC]),
        wi_e=wt(f("W_ih_e")), wh_e=wt(f("W_hh_e")),
        wi_d=wt(f("W_ih_d")), wh_d=wt(f("W_hh_d")),
        bias_e=bias(f("b_ih_e"), f("b_hh_e")),
        bias_d=bias(f("b_ih_d"), f("b_hh_d")),
        mask=mask, ind=ind,
    )
    in_maps = []
    for c in range(NCORES):
        ws = np.zeros((VSP, H), np.float32)
        ws[:VSH] = W_out[c * VSH:(c + 1) * VSH] * WSCALE
        wot8 = np.ascontiguousarray(
            ws.T.reshape(KC, 128, VSP).transpose(1, 0, 2)).astype(FP8)
        # bias pre-scaled to the psum scale, broadcast over partitions
        bx = np.full(VSP, -88.0 * WSCALE * HSCALE, np.float32)
        bx[:VSH] = b_out[c * VSH:(c + 1) * VSH] * (WSCALE * HSCALE)
        bout = np.ascontiguousarray(
            np.broadcast_to(bx, (128, VSP))).astype(BF16)
        in_maps.append(dict(common, wot8=wot8, bout=bout))

    tgt_next = tl[1:TGT].reshape(-1)                     # [3008]
    w_tgt = W_out[tgt_next]                              # [3008, 512]
    b_tgt = b_out[tgt_next].astype(np.float64)
    return in_maps, (w_tgt, b_tgt)


def _combine(results, tgt_data):
    w_tgt, b_tgt = tgt_data
    s = np.zeros(((SB + 127) // 128) * 128, np.float64)
    for r in results:
        s += np.asarray(r["out_s"], np.float64).T.reshape(-1)
    lse = np.log(s[:SB])
    # l_tgt = h . W_out[tgt] + b[tgt] from the DMA'd decoder h (core 0)
    hT = np.asarray(results[0]["out_h"], np.float32).reshape(128, KC, SBP)
    h = hT[:, :, :SB].transpose(2, 1, 0).reshape(SB, H)  # [t*B, k*128+p]
    l_tgt = np.einsum("ij,ij->i", h, w_tgt.astype(np.float32),
                      dtype=np.float64) + b_tgt
    return np.float32((lse - l_tgt).sum() / B)


def kernel(**inputs):
    global _COMPILED
    from concourse.bass_utils import run_bass_kernel_spmd
    in_maps, tgt_data = _prep(inputs)
    if _COMPILED is None:
        _COMPILED = _build()
    res = run_bass_kernel_spmd(_COMPILED, in_maps, list(range(NCORES)))
    return _combine(res.results, tgt_data)


if __name__ == "__main__":
    import reference
    inp = reference.setup_inputs()
    expected = np.asarray(reference.reference(**inp))
    actual = kernel(**{k: np.asarray(v) for k, v in inp.items()})
    err = abs(actual - expected) / max(abs(expected), 1e-9)
    print(f"expected={expected} actual={actual} rel_err={err:.3e}")
